# revision 1
# baseline (speedup 1.0000x reference)
"""HRR adapted attention kernel for 8 trn2 cores — frequency-sharded.

Math (same as baseline, verified in numpy):
  q,k,v = h @ W{q,k,v}.T + b      (per-row, D=2048)
  Qf = rfft(q); Kf = rfft(k)/(|rfft(k)|+eps); Vf likewise
  Mf = causal-cumsum_S(Kf*Vf);  Of = conj(Qf)*Mf;  adapter = irfft(Of)
  out = base + gate*adapter

Sharding: each core owns 128 of the 1024 packed rfft bins and processes
ALL B*S=8192 rows for those bins.  The DFT folds into the projections
per-core (G = W.T @ C_slice), so there is NO AllGather of the folded
weights and NO cross-core cumsum exchange — the causal scan runs fully
on-core with tensor_tensor_scan carry chaining.  The only collective is
a small per-chunk ReduceScatter of the irfft partial sums (each core
ends with its own D/8 output columns for all rows).

Packed spectrum: row 0 re-plane = DC, row 0 im-plane = Nyquist (both
real bins, core 0 only); handled SPMD-uniformly via per-core mask
vectors instead of code branches.
"""

import numpy as np
import ml_dtypes

import concourse.bass as bass
import concourse.mybir as mybir
import concourse.tile as tile
from concourse import bacc, bass_utils
from concourse.masks import make_identity

F32 = mybir.dt.float32
BF16 = mybir.dt.bfloat16
ALU = mybir.AluOpType
ACTF = mybir.ActivationFunctionType

B, S, D = 2, 4096, 2048
R = B * S                  # 8192 flat rows
NCORES = 8
FP = D // 2                # 1024 packed rfft bins
FPC = FP // NCORES         # 128 bins per core
CH = 512                   # rows per chunk
NCH = R // CH              # 16 chunks
NE = D // 128              # 16 contraction tiles
DC = D // NCORES           # 256 output d-columns per core
EPS = 1e-8
BF = ml_dtypes.bfloat16

_CACHE = {}


def _build():
    nc = bacc.Bacc("TRN2", target_bir_lowering=False, debug=False,
                   enable_asserts=False, num_devices=NCORES)

    h_in = nc.dram_tensor("h", [R, D], BF16, kind="ExternalInput").ap()
    w_ins = [nc.dram_tensor(f"w{x}", [D, D], BF16, kind="ExternalInput").ap()
             for x in "kvq"]
    cm_in = nc.dram_tensor("cm", [D, 2 * FPC], BF16, kind="ExternalInput").ap()
    am_in = nc.dram_tensor("am", [FPC, D], BF16, kind="ExternalInput").ap()
    bm_in = nc.dram_tensor("bm", [FPC, D], BF16, kind="ExternalInput").ap()
    bf_in = nc.dram_tensor("bfc", [FPC, 6], F32, kind="ExternalInput").ap()
    base_in = nc.dram_tensor("base", [DC, R], BF16, kind="ExternalInput").ap()
    out_t = nc.dram_tensor("out", [DC, R], F32, kind="ExternalOutput").ap()

    with tile.TileContext(nc) as tc, \
         tc.tile_pool(name="pc", bufs=1) as PC, \
         tc.tile_pool(name="pm", bufs=2) as PM, \
         tc.tile_pool(name="pt", bufs=1) as PT, \
         tc.tile_pool(name="pt2", bufs=2) as PT2, \
         tc.tile_pool(name="pev", bufs=3) as PEV, \
         tc.tile_pool(name="dram", bufs=1, space="DRAM") as DR:

        # ---------- constants ----------
        bf_sb = PC.tile([128, 6], F32, tag="bf")
        nc.sync.dma_start(bf_sb[:], bf_in[:])
        eps_sb = PC.tile([128, 1], F32, tag="eps")
        nc.vector.memset(eps_sb[:], EPS * EPS)
        zeros_sb = PC.tile([128, CH], F32, tag="zeros")
        nc.vector.memset(zeros_sb[:], 0.0)

        # ---------- DRAM intermediates ----------
        NBLK = NCH // 4
        part = [DR.tile([D, 4 * CH], BF16, tag=f"part{b}", name=f"part{b}")
                for b in range(NBLK)]
        rsout = [DR.tile([DC, 4 * CH], BF16, tag=f"rso{b}", name=f"rso{b}")
                 for b in range(NBLK)]

        # ---------- fold: G[w] = W.T @ [C|S] for this core's bins ----------
        G = [[PC.tile([128, 2 * FPC], BF16, tag=f"G{w}_{et}", name=f"G{w}_{et}")
              for et in range(NE)] for w in range(3)]
        state = {}
        PRE_HT = 2

        def load_hT(c):
            r0 = c * CH
            hT = PM.tile([128, NE * CH], BF16, tag="hT", name="hT", bufs=2)
            nc.sync.dma_start_transpose(
                hT[:].rearrange("p (t s) -> p t s", t=NE),
                h_in[r0:r0 + CH, :])
            state[("hT", c)] = hT

        with tc.tile_pool(name="pfold", bufs=4) as PF, \
             tc.tile_pool(name="pfoldc", bufs=1) as PFC, \
             tc.tile_pool(name="pfoldp", bufs=1, space="PSUM") as PFP:
            cm_sb = PFC.tile([128, NE * 2 * FPC], BF16, tag="cm")
            nc.sync.dma_start(
                cm_sb[:].rearrange("p (t f) -> p t f", t=NE),
                cm_in.rearrange("(t p) f -> p t f", p=128))
            for c0 in range(PRE_HT):
                load_hT(c0)
            pg = [PFP.tile([128, 2 * FPC], F32, tag=f"pg{et}", name=f"pg{et}")
                  for et in range(NE // 2)]
            EH = D // 2            # e-columns per half pass
            for w in range(3):
                for eh in range(2):
                    for dt in range(NE):
                        st = PF.tile([128, EH], BF16, tag="wstage",
                                     name="wstage")
                        nc.sync.dma_start(
                            st[:], w_ins[w][dt * 128:(dt + 1) * 128,
                                            eh * EH:(eh + 1) * EH])
                        for et in range(NE // 2):
                            nc.tensor.matmul(
                                pg[et][:], st[:, et * 128:(et + 1) * 128],
                                cm_sb[:, dt * 2 * FPC:(dt + 1) * 2 * FPC],
                                start=(dt == 0), stop=(dt == NE - 1))
                    for et in range(NE // 2):
                        dst = G[w][eh * (NE // 2) + et][:]
                        if et % 2 == 0:
                            nc.scalar.copy(dst, pg[et][:])
                        else:
                            nc.vector.tensor_copy(dst, pg[et][:])

        a_sb = PC.tile([128, D], BF16, tag="a_sb")
        nc.sync.dma_start(a_sb[:], am_in[:])
        b_sb = PC.tile([128, D], BF16, tag="b_sb")
        nc.sync.dma_start(b_sb[:], bm_in[:])

        PPX = tc.tile_pool(name="psum", bufs=1, space="PSUM")
        PP = PPX.__enter__()

        def proj_bind(c):
            r0 = c * CH
            hT = state.pop(("hT", c))
            planes = []
            for mi in range(6):
                w, hf = mi // 2, mi % 2
                ps = PP.tile([128, CH], F32, tag=f"pp{mi % 2}", name="pp", bufs=2)
                for et in range(NE):
                    nc.tensor.matmul(
                        ps[:], G[w][et][:, hf * FPC:(hf + 1) * FPC],
                        hT[:, et * CH:(et + 1) * CH],
                        start=(et == 0), stop=(et == NE - 1))
                pl = PM.tile([128, CH], F32, tag=f"pl{mi}", name=f"pl{mi}")
                if mi % 2 == 0:
                    nc.scalar.activation(pl[:], ps[:], ACTF.Identity,
                                         bias=bf_sb[:, mi:mi + 1])
                else:
                    nc.vector.tensor_scalar_add(pl[:], ps[:],
                                                bf_sb[:, mi:mi + 1])
                planes.append(pl)
            kre, kim, vre, vim, qre, qim = planes

            def T(tg):
                return PT.tile([128, CH], F32, tag=tg, name=tg)

            t1, t2 = T("t1"), T("t2")
            rk, rv = T("rk"), T("rv")
            nc.scalar.square(t1[:], kre[:])
            nc.scalar.square(t2[:], kim[:])
            nc.vector.tensor_add(rk[:], t1[:], t2[:])
            nc.scalar.square(t1[:], vre[:])
            nc.scalar.square(t2[:], vim[:])
            nc.vector.tensor_add(rv[:], t1[:], t2[:])
            nc.vector.tensor_mul(rk[:], rk[:], rv[:])
            nc.scalar.activation(rk[:], rk[:], ACTF.Sqrt, bias=eps_sb[:])
            nc.vector.reciprocal(rk[:], rk[:])
            cre, cim = T("cre"), T("cim")
            nc.vector.tensor_mul(t1[:], kre[:], vre[:])
            nc.vector.tensor_mul(t2[:], kim[:], vim[:])
            nc.vector.tensor_sub(cre[:], t1[:], t2[:])
            nc.vector.tensor_mul(t1[:], kre[:], vim[:])
            nc.vector.tensor_mul(t2[:], kim[:], vre[:])
            nc.vector.tensor_add(cim[:], t1[:], t2[:])
            nc.vector.tensor_mul(cre[:], cre[:], rk[:])
            nc.vector.tensor_mul(cim[:], cim[:], rk[:])
            # causal scan; carry chains across chunks, resets per batch
            mre = PM.tile([128, CH], F32, tag="mre", name="mre")
            mim = PM.tile([128, CH], F32, tag="mim", name="mim")
            if c % (NCH // B) == 0:
                ire, iim = 0.0, 0.0
            else:
                pmre, pmim = state["m"]
                ire, iim = pmre[:, CH - 1:CH], pmim[:, CH - 1:CH]
            nc.vector.tensor_tensor_scan(mre[:], cre[:], zeros_sb[:], ire,
                                         ALU.add, ALU.add)
            nc.vector.tensor_tensor_scan(mim[:], cim[:], zeros_sb[:], iim,
                                         ALU.add, ALU.add)
            state["m"] = (mre, mim)
            # unbind: of = conj(q) * m, with row-0 fixup as above
            orf, oif = T("orf"), T("oif")
            nc.vector.tensor_mul(t1[:], qre[:], mre[:])
            nc.vector.tensor_mul(t2[:], qim[:], mim[:])
            nc.vector.tensor_add(orf[:], t1[:], t2[:])
            nc.vector.tensor_mul(t1[:], qre[:], mim[:])
            nc.vector.tensor_mul(t2[:], qim[:], mre[:])
            nc.vector.tensor_sub(oif[:], t1[:], t2[:])
            oreb = PM.tile([128, CH], BF16, tag="oreb", name="oreb")
            oimb = PM.tile([128, CH], BF16, tag="oimb", name="oimb")
            nc.scalar.copy(oreb[:], orf[:])
            nc.scalar.copy(oimb[:], oif[:])
            state[("of", c)] = (oreb, oimb)

        def irfft_rs(c):
            b, ci = c // 4, c % 4
            oreb, oimb = state.pop(("of", c))
            for half in range(2):
                stg = PEV.tile([128, 8 * CH], BF16, tag=f"pstg{half}",
                               name=f"pstg{half}", bufs=3)
                for j in range(8):
                    dt = half * 8 + j
                    pi = PP.tile([128, CH], F32, tag="pirf", name="pirf",
                                 bufs=4)
                    nc.tensor.matmul(pi[:], a_sb[:, dt * 128:(dt + 1) * 128],
                                     oreb[:], start=True, stop=False)
                    nc.tensor.matmul(pi[:], b_sb[:, dt * 128:(dt + 1) * 128],
                                     oimb[:], start=False, stop=True)
                    dst = stg[:, j * CH:(j + 1) * CH]
                    if dt % 2 == 0:
                        nc.vector.tensor_copy(dst, pi[:])
                    else:
                        nc.scalar.copy(dst, pi[:])
                nc.scalar.dma_start(
                    part[b][half * 8 * 128:(half + 1) * 8 * 128,
                            ci * CH:(ci + 1) * CH]
                    .rearrange("(t p) s -> p t s", p=128),
                    stg[:].rearrange("p (t s) -> p t s", t=8))
            if ci == 3:
                nc.gpsimd.collective_compute(
                    "ReduceScatter", ALU.add,
                    replica_groups=[list(range(NCORES))],
                    ins=[part[b].opt()], outs=[rsout[b].opt()])

        def epi(c):
            r0 = c * CH
            b, ci = c // 4, c % 4
            rsb = PT2.tile([128, 2 * CH], BF16, tag="rssb", name="rssb")
            nc.gpsimd.dma_start(
                rsb[:].rearrange("p (t s) -> p t s", t=2),
                rsout[b][:, ci * CH:(ci + 1) * CH]
                .rearrange("(t p) s -> p t s", p=128))
            btile = PT2.tile([128, 2 * CH], BF16, tag="btile", name="btile")
            nc.gpsimd.dma_start(
                btile[:].rearrange("p (t s) -> p t s", t=2),
                base_in[:, r0:r0 + CH].rearrange("(t p) s -> p t s", p=128))
            outb = PT2.tile([128, 2 * CH], F32, tag="outb", name="outb",
                            bufs=4)
            nc.gpsimd.tensor_tensor(outb[:], rsb[:], btile[:], ALU.add)
            nc.gpsimd.dma_start(
                out_t[:, r0:r0 + CH].rearrange("(t p) s -> p t s", p=128),
                outb[:].rearrange("p (t s) -> p t s", t=2))

        for it in range(NCH + 10):
            if it + PRE_HT < NCH:
                load_hT(it + PRE_HT)
            if it < NCH:
                proj_bind(it)
            if 1 <= it <= NCH:
                irfft_rs(it - 1)
            if it >= 10:
                epi(it - 10)
        PPX.__exit__(None, None, None)

    nc.compile()
    return nc


def _constants():
    d = np.arange(D, dtype=np.float64)
    e = np.arange(D, dtype=np.float64)
    cms, ams, bms = [], [], []
    for c in range(NCORES):
        js = np.arange(c * FPC, (c + 1) * FPC, dtype=np.float64)
        ang = 2.0 * np.pi * np.outer(d, js) / D
        cm = np.concatenate([np.cos(ang), -np.sin(ang)], axis=1)
        am = (2.0 / D) * np.cos(2.0 * np.pi * np.outer(js, e) / D)
        bm = -(2.0 / D) * np.sin(2.0 * np.pi * np.outer(js, e) / D)
        if c == 0:
            # DC and Nyquist are handled exactly on the host (folded into
            # base); row/col 0 contributes nothing on-device.
            cm[:, 0] = 0.0
            cm[:, FPC] = 0.0
            am[0, :] = 0.0
            bm[0, :] = 0.0
        cms.append(cm.astype(BF))
        ams.append(am.astype(BF))
        bms.append(bm.astype(BF))
    return cms, ams, bms


def _run(inputs, trace=False):
    if "nc" not in _CACHE:
        _CACHE["nc"] = _build()
    nc = _CACHE["nc"]
    cms, ams, bms = _CACHE.setdefault("const", _constants())

    h = np.ascontiguousarray(
        np.asarray(inputs["hidden_states"], np.float32).reshape(R, D)).astype(BF)
    base = np.ascontiguousarray(
        np.asarray(inputs["base_output"], np.float32).reshape(R, D))
    gate = np.asarray(inputs["gate"], np.float32).reshape(-1)[0]
    ws = {x: np.asarray(inputs[f"W{x}"], np.float32).astype(BF) for x in "qkv"}

    bfc = np.zeros((FP, 6), np.float64)
    for j, bn in enumerate(("bk", "bv", "bq")):
        spec = np.fft.rfft(np.asarray(inputs[bn], np.float64))
        bfc[:, 2 * j] = spec.real[:FP]
        bfc[:, 2 * j + 1] = spec.imag[:FP]
        bfc[0, 2 * j] = 0.0
        bfc[0, 2 * j + 1] = 0.0
    bfc = bfc.astype(np.float32)

    # Exact host-side handling of the two real bins (DC, Nyquist): their
    # adapter contribution is rank-1 over d and is folded into base.
    h64 = np.asarray(inputs["hidden_states"], np.float64).reshape(R, D)
    sgn = np.cos(np.pi * np.arange(D))            # (-1)^d
    spec_q = np.fft.rfft(np.asarray(inputs["bq"], np.float64))
    spec_k = np.fft.rfft(np.asarray(inputs["bk"], np.float64))
    spec_v = np.fft.rfft(np.asarray(inputs["bv"], np.float64))
    w64 = {x: np.asarray(inputs[f"W{x}"], np.float64) for x in "qkv"}
    corr = np.zeros((R, D), np.float64)
    for bin_idx, fold in ((0, np.ones(D)), (FP, sgn)):
        gq = w64["q"].T @ fold
        gk = w64["k"].T @ fold
        gv = w64["v"].T @ fold
        qb = h64 @ gq + (spec_q.real[bin_idx])
        kb = h64 @ gk + (spec_k.real[bin_idx])
        vb = h64 @ gv + (spec_v.real[bin_idx])
        kb = kb / (np.abs(kb) + EPS)
        vb = vb / (np.abs(vb) + EPS)
        mem = np.cumsum((kb * vb).reshape(B, S), axis=1).reshape(R)
        ob = qb * mem / D                          # w=1 for real bins
        corr += np.outer(ob, fold)
    gate64 = float(np.asarray(inputs["gate"], np.float64).reshape(-1)[0])
    base = base + (gate64 * corr).astype(np.float32)

    in_maps = []
    for c in range(NCORES):
        in_maps.append({
            "h": h,
            "wk": ws["k"], "wv": ws["v"], "wq": ws["q"],
            "cm": cms[c],
            "am": (ams[c].astype(np.float32) * gate).astype(BF),
            "bm": (bms[c].astype(np.float32) * gate).astype(BF),
            "bfc": np.ascontiguousarray(bfc[c * FPC:(c + 1) * FPC]),
            "base": np.ascontiguousarray(base[:, c * DC:(c + 1) * DC].T).astype(BF),
        })

    res = bass_utils.run_bass_kernel_spmd(
        nc, in_maps, core_ids=list(range(NCORES)), trace=trace)
    out = np.concatenate(
        [np.asarray(res.results[c]["out"]) for c in range(NCORES)], axis=0)
    # restore the bf16 quantization of base exactly (host-side residual)
    full = np.ascontiguousarray(out.T).astype(np.float32)
    full += base - base.astype(BF).astype(np.float32)
    return full.reshape(B, S, D), res


def kernel(**inputs) -> np.ndarray:
    out, _ = _run(inputs, trace=False)
    return out



# revision 2
# speedup vs baseline: 1.3497x; 1.3497x over previous
"""HRR adapted attention kernel for 8 trn2 cores — frequency-sharded.

Math (verified in numpy):
  q,k,v = h @ W{q,k,v}.T + b      (per-row, D=2048)
  Qf = rfft(q); Kf = rfft(k)/(|rfft(k)|+eps); Vf likewise
  Mf = causal-cumsum_S(Kf*Vf);  Of = conj(Qf)*Mf;  adapter = irfft(Of)
  out = base + gate*adapter

Sharding: each core owns 128 of the 1024 packed rfft bins and processes
ALL B*S=8192 rows for those bins.  The DFT-folded projections
G = W.T @ C_slice are computed on the HOST (exact f32 fold, then bf16
quantize) so the device only runs the [256-spec x 2048-d] x rows
projection, the bind/scan/unbind vector chain, and the irfft partials.
h is pre-transposed on the host to [D, R] so hT chunk loads are plain
DMAs.  The causal scan runs fully on-core with tensor_tensor_scan carry
chaining.  The only collective is a per-block ReduceScatter of the
irfft partial sums (each core ends with its own D/8 output columns);
blocks are uneven [6,6,3,1] chunks so the final RS is small and the
drain tail short.

Packed spectrum: row 0 re-plane = DC, row 0 im-plane = Nyquist (both
real bins); their rank-1 contribution is folded into base on the host.
"""

import numpy as np
import ml_dtypes

import concourse.bass as bass
import concourse.mybir as mybir
import concourse.tile as tile
from concourse import bacc, bass_utils

F32 = mybir.dt.float32
BF16 = mybir.dt.bfloat16
ALU = mybir.AluOpType
ACTF = mybir.ActivationFunctionType

B, S, D = 2, 4096, 2048
R = B * S                  # 8192 flat rows
NCORES = 8
FP = D // 2                # 1024 packed rfft bins
FPC = FP // NCORES         # 128 bins per core
CH = 512                   # rows per chunk
NCH = R // CH              # 16 chunks
NE = D // 128              # 16 contraction tiles
DC = D // NCORES           # 256 output d-columns per core
EPS = 1e-8
BF = ml_dtypes.bfloat16

# uneven ReduceScatter blocks (in chunks): big early blocks fully
# overlap compute; the tiny final block keeps the drain tail short.
BLOCK_SIZES = [6, 6, 3, 1]
BLOCK_ENDS = list(np.cumsum(BLOCK_SIZES) - 1)        # [5, 11, 14, 15]
BLOCK_STARTS = [e - s + 1 for e, s in zip(BLOCK_ENDS, BLOCK_SIZES)]
CHUNK_BLOCK = {}
for _b, (_s, _e) in enumerate(zip(BLOCK_STARTS, BLOCK_ENDS)):
    for _c in range(_s, _e + 1):
        CHUNK_BLOCK[_c] = _b
NBLK = len(BLOCK_SIZES)
EPI_LAG = 7

_CACHE = {}


def _build():
    nc = bacc.Bacc("TRN2", target_bir_lowering=False, debug=False,
                   enable_asserts=False, num_devices=NCORES)

    h_in = nc.dram_tensor("h", [D, R], BF16, kind="ExternalInput").ap()
    g_ins = [nc.dram_tensor(f"g{x}", [D, 2 * FPC], BF16,
                            kind="ExternalInput").ap() for x in "kvq"]
    am_in = nc.dram_tensor("am", [FPC, D], BF16, kind="ExternalInput").ap()
    bm_in = nc.dram_tensor("bm", [FPC, D], BF16, kind="ExternalInput").ap()
    bf_in = nc.dram_tensor("bfc", [FPC, 6], F32, kind="ExternalInput").ap()
    base_in = nc.dram_tensor("base", [DC, R], BF16, kind="ExternalInput").ap()
    out_t = nc.dram_tensor("out", [DC, R], F32, kind="ExternalOutput").ap()

    with tile.TileContext(nc) as tc, \
         tc.tile_pool(name="pc", bufs=1) as PC, \
         tc.tile_pool(name="pm", bufs=2) as PM, \
         tc.tile_pool(name="pt", bufs=1) as PT, \
         tc.tile_pool(name="pt2", bufs=2) as PT2, \
         tc.tile_pool(name="pev", bufs=3) as PEV, \
         tc.tile_pool(name="psum", bufs=1, space="PSUM") as PP, \
         tc.tile_pool(name="dram", bufs=1, space="DRAM") as DR:

        state = {}

        def load_hT(c):
            r0 = c * CH
            hT = PM.tile([128, NE * CH], BF16, tag="hT", name="hT", bufs=3)
            nc.sync.dma_start(
                hT[:].rearrange("p (t s) -> p t s", t=NE),
                h_in[:, r0:r0 + CH].rearrange("(t p) s -> p t s", p=128))
            state[("hT", c)] = hT

        # ---------- startup loads: hT0 first (first compute dependency),
        # then the folded projections, constants, and the prefetch hTs.
        load_hT(0)
        G = []
        for w in range(3):
            g_sb = PC.tile([128, NE * 2 * FPC], BF16, tag=f"G{w}",
                           name=f"G{w}")
            nc.sync.dma_start(
                g_sb[:].rearrange("p (t f) -> p t f", t=NE),
                g_ins[w].rearrange("(t p) f -> p t f", p=128))
            G.append(g_sb)
        bf_sb = PC.tile([128, 6], F32, tag="bf")
        nc.sync.dma_start(bf_sb[:], bf_in[:])
        eps_sb = PC.tile([128, 1], F32, tag="eps")
        nc.vector.memset(eps_sb[:], EPS * EPS)
        zeros_sb = PC.tile([128, CH], F32, tag="zeros")
        nc.vector.memset(zeros_sb[:], 0.0)
        a_sb = PC.tile([128, D], BF16, tag="a_sb")
        nc.scalar.dma_start(a_sb[:], am_in[:])
        b_sb = PC.tile([128, D], BF16, tag="b_sb")
        nc.scalar.dma_start(b_sb[:], bm_in[:])
        load_hT(1)
        PRE_HT = 2

        # ---------- DRAM intermediates ----------
        part = [DR.tile([D, BLOCK_SIZES[b] * CH], BF16, tag=f"part{b}",
                        name=f"part{b}") for b in range(NBLK)]
        rsout = [DR.tile([DC, BLOCK_SIZES[b] * CH], BF16, tag=f"rso{b}",
                         name=f"rso{b}") for b in range(NBLK)]

        def proj_bind(c):
            hT = state.pop(("hT", c))
            planes = []
            for mi in range(6):
                w, hf = mi // 2, mi % 2
                ps = PP.tile([128, CH], F32, tag=f"pp{mi % 2}", name="pp",
                             bufs=2)
                for et in range(NE):
                    nc.tensor.matmul(
                        ps[:],
                        G[w][:, et * 2 * FPC + hf * FPC:
                             et * 2 * FPC + (hf + 1) * FPC],
                        hT[:, et * CH:(et + 1) * CH],
                        start=(et == 0), stop=(et == NE - 1))
                pl = PM.tile([128, CH], F32, tag=f"pl{mi}", name=f"pl{mi}")
                if mi % 2 == 0:
                    nc.scalar.activation(pl[:], ps[:], ACTF.Identity,
                                         bias=bf_sb[:, mi:mi + 1])
                else:
                    nc.vector.tensor_scalar_add(pl[:], ps[:],
                                                bf_sb[:, mi:mi + 1])
                planes.append(pl)
            kre, kim, vre, vim, qre, qim = planes

            def T(tg):
                return PT.tile([128, CH], F32, tag=tg, name=tg)

            t1, t2 = T("t1"), T("t2")
            rk, rv = T("rk"), T("rv")
            nc.scalar.square(t1[:], kre[:])
            nc.scalar.square(t2[:], kim[:])
            nc.vector.tensor_add(rk[:], t1[:], t2[:])
            nc.scalar.square(t1[:], vre[:])
            nc.scalar.square(t2[:], vim[:])
            nc.vector.tensor_add(rv[:], t1[:], t2[:])
            nc.vector.tensor_mul(rk[:], rk[:], rv[:])
            nc.scalar.activation(rk[:], rk[:], ACTF.Sqrt, bias=eps_sb[:])
            nc.vector.reciprocal(rk[:], rk[:])
            cre, cim = T("cre"), T("cim")
            nc.vector.tensor_mul(t1[:], kre[:], vre[:])
            nc.vector.tensor_mul(t2[:], kim[:], vim[:])
            nc.vector.tensor_sub(cre[:], t1[:], t2[:])
            nc.vector.tensor_mul(t1[:], kre[:], vim[:])
            nc.vector.tensor_mul(t2[:], kim[:], vre[:])
            nc.vector.tensor_add(cim[:], t1[:], t2[:])
            nc.vector.tensor_mul(cre[:], cre[:], rk[:])
            nc.vector.tensor_mul(cim[:], cim[:], rk[:])
            # causal scan; carry chains across chunks, resets per batch
            mre = PM.tile([128, CH], F32, tag="mre", name="mre")
            mim = PM.tile([128, CH], F32, tag="mim", name="mim")
            if c % (NCH // B) == 0:
                ire, iim = 0.0, 0.0
            else:
                pmre, pmim = state["m"]
                ire, iim = pmre[:, CH - 1:CH], pmim[:, CH - 1:CH]
            nc.vector.tensor_tensor_scan(mre[:], cre[:], zeros_sb[:], ire,
                                         ALU.add, ALU.add)
            nc.vector.tensor_tensor_scan(mim[:], cim[:], zeros_sb[:], iim,
                                         ALU.add, ALU.add)
            state["m"] = (mre, mim)
            # unbind: of = conj(q) * m
            orf, oif = T("orf"), T("oif")
            nc.vector.tensor_mul(t1[:], qre[:], mre[:])
            nc.vector.tensor_mul(t2[:], qim[:], mim[:])
            nc.vector.tensor_add(orf[:], t1[:], t2[:])
            nc.vector.tensor_mul(t1[:], qre[:], mim[:])
            nc.vector.tensor_mul(t2[:], qim[:], mre[:])
            nc.vector.tensor_sub(oif[:], t1[:], t2[:])
            oreb = PM.tile([128, CH], BF16, tag="oreb", name="oreb")
            oimb = PM.tile([128, CH], BF16, tag="oimb", name="oimb")
            nc.scalar.copy(oreb[:], orf[:])
            nc.scalar.copy(oimb[:], oif[:])
            state[("of", c)] = (oreb, oimb)

        def irfft_rs(c):
            b = CHUNK_BLOCK[c]
            ci = c - BLOCK_STARTS[b]
            oreb, oimb = state.pop(("of", c))
            for half in range(2):
                stg = PEV.tile([128, 8 * CH], BF16, tag=f"pstg{half}",
                               name=f"pstg{half}", bufs=3)
                for j in range(8):
                    dt = half * 8 + j
                    pi = PP.tile([128, CH], F32, tag="pirf", name="pirf",
                                 bufs=4)
                    nc.tensor.matmul(pi[:], a_sb[:, dt * 128:(dt + 1) * 128],
                                     oreb[:], start=True, stop=False)
                    nc.tensor.matmul(pi[:], b_sb[:, dt * 128:(dt + 1) * 128],
                                     oimb[:], start=False, stop=True)
                    dst = stg[:, j * CH:(j + 1) * CH]
                    if dt % 2 == 0:
                        nc.vector.tensor_copy(dst, pi[:])
                    else:
                        nc.scalar.copy(dst, pi[:])
                nc.scalar.dma_start(
                    part[b][half * 8 * 128:(half + 1) * 8 * 128,
                            ci * CH:(ci + 1) * CH]
                    .rearrange("(t p) s -> p t s", p=128),
                    stg[:].rearrange("p (t s) -> p t s", t=8))
            if c == BLOCK_ENDS[b]:
                nc.gpsimd.collective_compute(
                    "ReduceScatter", ALU.add,
                    replica_groups=[list(range(NCORES))],
                    ins=[part[b].opt()], outs=[rsout[b].opt()])

        def epi(c):
            r0 = c * CH
            b = CHUNK_BLOCK[c]
            ci = c - BLOCK_STARTS[b]
            rsb = PT2.tile([128, 2 * CH], BF16, tag="rssb", name="rssb")
            nc.gpsimd.dma_start(
                rsb[:].rearrange("p (t s) -> p t s", t=2),
                rsout[b][:, ci * CH:(ci + 1) * CH]
                .rearrange("(t p) s -> p t s", p=128))
            btile = PT2.tile([128, 2 * CH], BF16, tag="btile", name="btile")
            nc.gpsimd.dma_start(
                btile[:].rearrange("p (t s) -> p t s", t=2),
                base_in[:, r0:r0 + CH].rearrange("(t p) s -> p t s", p=128))
            outb = PT2.tile([128, 2 * CH], F32, tag="outb", name="outb",
                            bufs=4)
            nc.gpsimd.tensor_tensor(outb[:], rsb[:], btile[:], ALU.add)
            nc.gpsimd.dma_start(
                out_t[:, r0:r0 + CH].rearrange("(t p) s -> p t s", p=128),
                outb[:].rearrange("p (t s) -> p t s", t=2))

        for it in range(NCH + EPI_LAG + 1):
            if it + PRE_HT < NCH:
                load_hT(it + PRE_HT)
            if it < NCH:
                proj_bind(it)
            if 1 <= it <= NCH:
                irfft_rs(it - 1)
            if it >= EPI_LAG and it - EPI_LAG < NCH:
                epi(it - EPI_LAG)

    nc.compile()
    return nc


def _constants():
    """Cached: DFT fold matrix C (f32, [D, 2*FP] per-core block layout),
    irfft matrices am/bm per core (f64), and the (-1)^d vector."""
    d = np.arange(D, dtype=np.float64)
    e = np.arange(D, dtype=np.float64)
    cs, ams, bms = [], [], []
    for c in range(NCORES):
        js = np.arange(c * FPC, (c + 1) * FPC, dtype=np.float64)
        ang = 2.0 * np.pi * np.outer(d, js) / D
        cm = np.concatenate([np.cos(ang), -np.sin(ang)], axis=1)
        am = (2.0 / D) * np.cos(2.0 * np.pi * np.outer(js, e) / D)
        bm = -(2.0 / D) * np.sin(2.0 * np.pi * np.outer(js, e) / D)
        if c == 0:
            # DC and Nyquist are handled exactly on the host (folded into
            # base); row/col 0 contributes nothing on-device.
            cm[:, 0] = 0.0
            cm[:, FPC] = 0.0
            am[0, :] = 0.0
            bm[0, :] = 0.0
        cs.append(cm)
        ams.append(am.astype(BF))
        bms.append(bm.astype(BF))
    cfull = np.ascontiguousarray(
        np.concatenate(cs, axis=1).astype(np.float32))   # [D, 8*2*FPC]
    return cfull, ams, bms


def _run(inputs, trace=False):
    if "nc" not in _CACHE:
        _CACHE["nc"] = _build()
    nc = _CACHE["nc"]
    cfull, ams, bms = _CACHE.setdefault("const", _constants())

    h32 = np.asarray(inputs["hidden_states"], np.float32).reshape(R, D)
    hT = np.ascontiguousarray(h32.T).astype(BF)          # [D, R]
    base = np.ascontiguousarray(
        np.asarray(inputs["base_output"], np.float32).reshape(R, D))
    gate = np.asarray(inputs["gate"], np.float32).reshape(-1)[0]

    # Host-side fold of the DFT into the projections: G = W.T @ C (f32,
    # exact to ~1e-6), then bf16 quantize.  Per-core slice = 256 cols.
    gfold = {}
    for x in "kvq":
        w = np.asarray(inputs[f"W{x}"], np.float32)
        gfold[x] = (w.T @ cfull).astype(BF)              # [D, 2*FP]

    bfc = np.zeros((FP, 6), np.float64)
    for j, bn in enumerate(("bk", "bv", "bq")):
        spec = np.fft.rfft(np.asarray(inputs[bn], np.float64))
        bfc[:, 2 * j] = spec.real[:FP]
        bfc[:, 2 * j + 1] = spec.imag[:FP]
        bfc[0, 2 * j] = 0.0
        bfc[0, 2 * j + 1] = 0.0
    bfc = bfc.astype(np.float32)

    # Exact host-side handling of the two real bins (DC, Nyquist): their
    # adapter contribution is rank-1 over d and is folded into base.
    h64 = np.asarray(inputs["hidden_states"], np.float64).reshape(R, D)
    sgn = np.cos(np.pi * np.arange(D))            # (-1)^d
    spec_q = np.fft.rfft(np.asarray(inputs["bq"], np.float64))
    spec_k = np.fft.rfft(np.asarray(inputs["bk"], np.float64))
    spec_v = np.fft.rfft(np.asarray(inputs["bv"], np.float64))
    w64 = {x: np.asarray(inputs[f"W{x}"], np.float64) for x in "qkv"}
    corr = np.zeros((R, D), np.float64)
    for bin_idx, fold in ((0, np.ones(D)), (FP, sgn)):
        gq = w64["q"].T @ fold
        gk = w64["k"].T @ fold
        gv = w64["v"].T @ fold
        qb = h64 @ gq + (spec_q.real[bin_idx])
        kb = h64 @ gk + (spec_k.real[bin_idx])
        vb = h64 @ gv + (spec_v.real[bin_idx])
        kb = kb / (np.abs(kb) + EPS)
        vb = vb / (np.abs(vb) + EPS)
        mem = np.cumsum((kb * vb).reshape(B, S), axis=1).reshape(R)
        ob = qb * mem / D                          # w=1 for real bins
        corr += np.outer(ob, fold)
    gate64 = float(np.asarray(inputs["gate"], np.float64).reshape(-1)[0])
    base = base + (gate64 * corr).astype(np.float32)

    in_maps = []
    for c in range(NCORES):
        sl = slice(c * 2 * FPC, (c + 1) * 2 * FPC)
        in_maps.append({
            "h": hT,
            "gk": np.ascontiguousarray(gfold["k"][:, sl]),
            "gv": np.ascontiguousarray(gfold["v"][:, sl]),
            "gq": np.ascontiguousarray(gfold["q"][:, sl]),
            "am": (ams[c].astype(np.float32) * gate).astype(BF),
            "bm": (bms[c].astype(np.float32) * gate).astype(BF),
            "bfc": np.ascontiguousarray(bfc[c * FPC:(c + 1) * FPC]),
            "base": np.ascontiguousarray(base[:, c * DC:(c + 1) * DC].T).astype(BF),
        })

    res = bass_utils.run_bass_kernel_spmd(
        nc, in_maps, core_ids=list(range(NCORES)), trace=trace)
    out = np.concatenate(
        [np.asarray(res.results[c]["out"]) for c in range(NCORES)], axis=0)
    # restore the bf16 quantization of base exactly (host-side residual)
    full = np.ascontiguousarray(out.T).astype(np.float32)
    full += base - base.astype(BF).astype(np.float32)
    return full.reshape(B, S, D), res


def kernel(**inputs) -> np.ndarray:
    out, _ = _run(inputs, trace=False)
    return out


# revision 3
# speedup vs baseline: 1.5420x; 1.1425x over previous
"""HRR adapted attention kernel for 8 trn2 cores — frequency-sharded.

Math (verified in numpy):
  q,k,v = h @ W{q,k,v}.T + b      (per-row, D=2048)
  Qf = rfft(q); Kf = rfft(k)/(|rfft(k)|+eps); Vf likewise
  Mf = causal-cumsum_S(Kf*Vf);  Of = conj(Qf)*Mf;  adapter = irfft(Of)
  out = base + gate*adapter

Sharding: each core owns 128 of the 1024 packed rfft bins and processes
ALL B*S=8192 rows for those bins.  The DFT-folded projections
G = W.T @ C_slice are computed on the HOST (exact, via rfft of W.T) and
shipped as a two-term fp8 split G = G1 + G2 (G2 the quantization
residual); h likewise as h1 + h2, pre-transposed to [D, R].  The device
projection then runs three fp8 DoubleRow matmul phases
(h1G1 + h1G2 + h2G1, dropping the negligible h2G2), which is both
faster than bf16 (DoubleRow contracts 256 per instruction at half
cycles/row) and slightly more accurate.  The bind/scan/unbind chain and
the bf16 irfft partials are unchanged.  The causal scan runs fully
on-core with tensor_tensor_scan carry chaining.  The only collective is
a per-block ReduceScatter of the irfft partial sums; blocks are
[6,6,1,1,1,1] chunks so the last chunks' collectives pipeline at chunk
rate and the drain tail stays short.

Packed spectrum: row 0 re-plane = DC, row 0 im-plane = Nyquist (both
real bins); their rank-1 contribution is folded into base on the host.
"""

import numpy as np
import ml_dtypes

import concourse.bass as bass
import concourse.mybir as mybir
import concourse.tile as tile
from concourse import bacc, bass_utils

F32 = mybir.dt.float32
BF16 = mybir.dt.bfloat16
F8 = mybir.dt.float8e4
ALU = mybir.AluOpType
ACTF = mybir.ActivationFunctionType
DROW = mybir.MatmulPerfMode.DoubleRow

B, S, D = 2, 4096, 2048
R = B * S                  # 8192 flat rows
NCORES = 8
FP = D // 2                # 1024 packed rfft bins
FPC = FP // NCORES         # 128 bins per core
CH = 512                   # rows per chunk
NCH = R // CH              # 16 chunks
NE = D // 128              # 16 contraction tiles
NE2 = NE // 2              # 8 DoubleRow contraction tiles
DC = D // NCORES           # 256 output d-columns per core
EPS = 1e-8
BF = ml_dtypes.bfloat16
F8NP = ml_dtypes.float8_e4m3fn

# ReduceScatter blocks (in chunks): big early blocks fully overlap
# compute; per-chunk blocks at the end pipeline at chunk rate so the
# drain tail is one small collective + one epilogue.
BLOCK_SIZES = [6, 6, 1, 1, 1, 1]
BLOCK_ENDS = list(np.cumsum(BLOCK_SIZES) - 1)
BLOCK_STARTS = [e - s + 1 for e, s in zip(BLOCK_ENDS, BLOCK_SIZES)]
CHUNK_BLOCK = {}
for _b, (_s, _e) in enumerate(zip(BLOCK_STARTS, BLOCK_ENDS)):
    for _c in range(_s, _e + 1):
        CHUNK_BLOCK[_c] = _b
NBLK = len(BLOCK_SIZES)
EPI_LAG = 7

_CACHE = {}


def _build():
    nc = bacc.Bacc("TRN2", target_bir_lowering=False, debug=False,
                   enable_asserts=False, num_devices=NCORES)

    h_ins = [nc.dram_tensor(f"h{i}", [D, R], F8, kind="ExternalInput").ap()
             for i in (1, 2)]
    g_ins = [[nc.dram_tensor(f"g{i}{x}", [D, 2 * FPC], F8,
                             kind="ExternalInput").ap() for x in "kvq"]
             for i in (1, 2)]
    am_in = nc.dram_tensor("am", [FPC, D], BF16, kind="ExternalInput").ap()
    bm_in = nc.dram_tensor("bm", [FPC, D], BF16, kind="ExternalInput").ap()
    bf_in = nc.dram_tensor("bfc", [FPC, 6], F32, kind="ExternalInput").ap()
    base_in = nc.dram_tensor("base", [DC, R], BF16, kind="ExternalInput").ap()
    out_t = nc.dram_tensor("out", [DC, R], F32, kind="ExternalOutput").ap()

    with tile.TileContext(nc) as tc, \
         tc.tile_pool(name="pc", bufs=1) as PC, \
         tc.tile_pool(name="pm", bufs=2) as PM, \
         tc.tile_pool(name="pt", bufs=1) as PT, \
         tc.tile_pool(name="pt2", bufs=2) as PT2, \
         tc.tile_pool(name="pev", bufs=3) as PEV, \
         tc.tile_pool(name="psum", bufs=1, space="PSUM") as PP, \
         tc.tile_pool(name="dram", bufs=1, space="DRAM") as DR:

        state = {}

        def load_hT(c, split=1):
            r0 = c * CH
            for i in (1, 2):
                hT = PM.tile([128, NE * CH], F8, tag=f"hT{i}",
                             name=f"hT{i}", bufs=3)
                tq = NE // split
                for q in range(split):
                    nc.sync.dma_start(
                        hT[:].rearrange("p (t s) -> p t s", t=NE)
                        [:, q * tq:(q + 1) * tq, :],
                        h_ins[i - 1][:, r0:r0 + CH]
                        .rearrange("(t p) s -> p t s", p=128)
                        [:, q * tq:(q + 1) * tq, :])
                state[(f"h{i}", c)] = hT

        def load_g(i, w, split=1):
            g_sb = PC.tile([128, NE * 2 * FPC], F8, tag=f"G{i}{w}",
                           name=f"G{i}{w}")
            tq = NE // split
            for q in range(split):
                nc.sync.dma_start(
                    g_sb[:].rearrange("p (t f) -> p t f", t=NE)
                    [:, q * tq:(q + 1) * tq, :],
                    g_ins[i - 1][w].rearrange("(t p) f -> p t f", p=128)
                    [:, q * tq:(q + 1) * tq, :])
            return g_sb

        # ---------- startup loads, dependency-ordered: the first matmul
        # phase (h1@G1k) only needs the first quarters of h1 and G1k.
        r0 = 0
        hT1c0 = PM.tile([128, NE * CH], F8, tag="hT1", name="hT1", bufs=3)
        nc.sync.dma_start(
            hT1c0[:].rearrange("p (t s) -> p t s", t=NE)[:, 0:4, :],
            h_ins[0][:, 0:CH].rearrange("(t p) s -> p t s", p=128)[:, 0:4, :])
        G1 = [None] * 3
        G2 = [None] * 3
        G1[0] = load_g(1, 0, split=4)
        bf_sb = PC.tile([128, 6], F32, tag="bf")
        nc.sync.dma_start(bf_sb[:], bf_in[:])
        for q in range(1, 4):
            nc.sync.dma_start(
                hT1c0[:].rearrange("p (t s) -> p t s", t=NE)
                [:, 4 * q:4 * (q + 1), :],
                h_ins[0][:, 0:CH].rearrange("(t p) s -> p t s", p=128)
                [:, 4 * q:4 * (q + 1), :])
        state[("h1", 0)] = hT1c0
        G2[0] = load_g(2, 0, split=2)
        hT2c0 = PM.tile([128, NE * CH], F8, tag="hT2", name="hT2", bufs=3)
        nc.sync.dma_start(
            hT2c0[:].rearrange("p (t s) -> p t s", t=NE),
            h_ins[1][:, 0:CH].rearrange("(t p) s -> p t s", p=128))
        state[("h2", 0)] = hT2c0
        for w in (1, 2):
            G1[w] = load_g(1, w)
            G2[w] = load_g(2, w)
        eps_sb = PC.tile([128, 1], F32, tag="eps")
        nc.vector.memset(eps_sb[:], EPS * EPS)
        zeros_sb = PC.tile([128, CH], F32, tag="zeros")
        nc.vector.memset(zeros_sb[:], 0.0)
        a_sb = PC.tile([128, D], BF16, tag="a_sb")
        nc.scalar.dma_start(a_sb[:], am_in[:])
        b_sb = PC.tile([128, D], BF16, tag="b_sb")
        nc.scalar.dma_start(b_sb[:], bm_in[:])
        load_hT(1)
        PRE_HT = 2

        # ---------- DRAM intermediates ----------
        part = [DR.tile([D, BLOCK_SIZES[b] * CH], BF16, tag=f"part{b}",
                        name=f"part{b}") for b in range(NBLK)]
        rsout = [DR.tile([DC, BLOCK_SIZES[b] * CH], BF16, tag=f"rso{b}",
                         name=f"rso{b}") for b in range(NBLK)]

        def proj_bind(c):
            h1 = state.pop(("h1", c))
            h2 = state.pop(("h2", c))
            phases = ((h1, G1), (h1, G2), (h2, G1))
            planes = []
            for mi in range(6):
                w, hf = mi // 2, mi % 2
                ps = PP.tile([128, CH], F32, tag=f"pp{mi % 2}", name="pp",
                             bufs=2)
                for pi, (hsrc, gsrc) in enumerate(phases):
                    g3 = gsrc[w][:].rearrange("p (t f) -> p t f", t=NE)
                    h3 = hsrc[:].rearrange("p (t s) -> p t s", t=NE)
                    for e2 in range(NE2):
                        nc.tensor.matmul(
                            ps[:],
                            g3[:, 2 * e2:2 * e2 + 2,
                               hf * FPC:(hf + 1) * FPC],
                            h3[:, 2 * e2:2 * e2 + 2, :],
                            start=(pi == 0 and e2 == 0),
                            stop=(pi == 2 and e2 == NE2 - 1),
                            perf_mode=DROW)
                pl = PM.tile([128, CH], F32, tag=f"pl{mi}", name=f"pl{mi}")
                if mi % 2 == 0:
                    nc.scalar.activation(pl[:], ps[:], ACTF.Identity,
                                         bias=bf_sb[:, mi:mi + 1])
                else:
                    nc.vector.tensor_scalar_add(pl[:], ps[:],
                                                bf_sb[:, mi:mi + 1])
                planes.append(pl)
            kre, kim, vre, vim, qre, qim = planes

            def T(tg):
                return PT.tile([128, CH], F32, tag=tg, name=tg)

            t1, t2 = T("t1"), T("t2")
            rk, rv = T("rk"), T("rv")
            nc.scalar.square(t1[:], kre[:])
            nc.scalar.square(t2[:], kim[:])
            nc.vector.tensor_add(rk[:], t1[:], t2[:])
            nc.scalar.square(t1[:], vre[:])
            nc.scalar.square(t2[:], vim[:])
            nc.vector.tensor_add(rv[:], t1[:], t2[:])
            nc.vector.tensor_mul(rk[:], rk[:], rv[:])
            nc.scalar.activation(rk[:], rk[:], ACTF.Sqrt, bias=eps_sb[:])
            nc.vector.reciprocal(rk[:], rk[:])
            cre, cim = T("cre"), T("cim")
            nc.vector.tensor_mul(t1[:], kre[:], vre[:])
            nc.vector.tensor_mul(t2[:], kim[:], vim[:])
            nc.vector.tensor_sub(cre[:], t1[:], t2[:])
            nc.vector.tensor_mul(t1[:], kre[:], vim[:])
            nc.vector.tensor_mul(t2[:], kim[:], vre[:])
            nc.vector.tensor_add(cim[:], t1[:], t2[:])
            nc.vector.tensor_mul(cre[:], cre[:], rk[:])
            nc.vector.tensor_mul(cim[:], cim[:], rk[:])
            # causal scan; carry chains across chunks, resets per batch
            mre = PM.tile([128, CH], F32, tag="mre", name="mre")
            mim = PM.tile([128, CH], F32, tag="mim", name="mim")
            if c % (NCH // B) == 0:
                ire, iim = 0.0, 0.0
            else:
                pmre, pmim = state["m"]
                ire, iim = pmre[:, CH - 1:CH], pmim[:, CH - 1:CH]
            nc.vector.tensor_tensor_scan(mre[:], cre[:], zeros_sb[:], ire,
                                         ALU.add, ALU.add)
            nc.vector.tensor_tensor_scan(mim[:], cim[:], zeros_sb[:], iim,
                                         ALU.add, ALU.add)
            state["m"] = (mre, mim)
            # unbind: of = conj(q) * m
            orf, oif = T("orf"), T("oif")
            nc.vector.tensor_mul(t1[:], qre[:], mre[:])
            nc.vector.tensor_mul(t2[:], qim[:], mim[:])
            nc.vector.tensor_add(orf[:], t1[:], t2[:])
            nc.vector.tensor_mul(t1[:], qre[:], mim[:])
            nc.vector.tensor_mul(t2[:], qim[:], mre[:])
            nc.vector.tensor_sub(oif[:], t1[:], t2[:])
            oreb = PM.tile([128, CH], BF16, tag="oreb", name="oreb")
            oimb = PM.tile([128, CH], BF16, tag="oimb", name="oimb")
            nc.scalar.copy(oreb[:], orf[:])
            nc.scalar.copy(oimb[:], oif[:])
            state[("of", c)] = (oreb, oimb)

        def irfft_rs(c):
            b = CHUNK_BLOCK[c]
            ci = c - BLOCK_STARTS[b]
            oreb, oimb = state.pop(("of", c))
            for half in range(2):
                stg = PEV.tile([128, 8 * CH], BF16, tag=f"pstg{half}",
                               name=f"pstg{half}", bufs=3)
                for j in range(8):
                    dt = half * 8 + j
                    pi = PP.tile([128, CH], F32, tag="pirf", name="pirf",
                                 bufs=4)
                    nc.tensor.matmul(pi[:], a_sb[:, dt * 128:(dt + 1) * 128],
                                     oreb[:], start=True, stop=False)
                    nc.tensor.matmul(pi[:], b_sb[:, dt * 128:(dt + 1) * 128],
                                     oimb[:], start=False, stop=True)
                    dst = stg[:, j * CH:(j + 1) * CH]
                    if dt % 2 == 0:
                        nc.vector.tensor_copy(dst, pi[:])
                    else:
                        nc.scalar.copy(dst, pi[:])
                nc.scalar.dma_start(
                    part[b][half * 8 * 128:(half + 1) * 8 * 128,
                            ci * CH:(ci + 1) * CH]
                    .rearrange("(t p) s -> p t s", p=128),
                    stg[:].rearrange("p (t s) -> p t s", t=8))
            if c == BLOCK_ENDS[b]:
                nc.gpsimd.collective_compute(
                    "ReduceScatter", ALU.add,
                    replica_groups=[list(range(NCORES))],
                    ins=[part[b].opt()], outs=[rsout[b].opt()])

        def epi(c):
            r0 = c * CH
            b = CHUNK_BLOCK[c]
            ci = c - BLOCK_STARTS[b]
            rsb = PT2.tile([128, 2 * CH], BF16, tag="rssb", name="rssb")
            nc.gpsimd.dma_start(
                rsb[:].rearrange("p (t s) -> p t s", t=2),
                rsout[b][:, ci * CH:(ci + 1) * CH]
                .rearrange("(t p) s -> p t s", p=128))
            btile = PT2.tile([128, 2 * CH], BF16, tag="btile", name="btile")
            nc.gpsimd.dma_start(
                btile[:].rearrange("p (t s) -> p t s", t=2),
                base_in[:, r0:r0 + CH].rearrange("(t p) s -> p t s", p=128))
            outb = PT2.tile([128, 2 * CH], F32, tag="outb", name="outb",
                            bufs=4)
            nc.gpsimd.tensor_tensor(outb[:], rsb[:], btile[:], ALU.add)
            nc.gpsimd.dma_start(
                out_t[:, r0:r0 + CH].rearrange("(t p) s -> p t s", p=128),
                outb[:].rearrange("p (t s) -> p t s", t=2))

        for it in range(NCH + EPI_LAG + 1):
            if it + PRE_HT < NCH:
                load_hT(it + PRE_HT)
            if it < NCH:
                proj_bind(it)
            if 1 <= it <= NCH:
                irfft_rs(it - 1)
            if it >= EPI_LAG and it - EPI_LAG < NCH:
                epi(it - EPI_LAG)

    nc.compile()
    return nc


def _constants():
    """Cached: per-core irfft matrices am/bm (bf16) and (-1)^d."""
    e = np.arange(D, dtype=np.float64)
    ams, bms = [], []
    for c in range(NCORES):
        js = np.arange(c * FPC, (c + 1) * FPC, dtype=np.float64)
        am = (2.0 / D) * np.cos(2.0 * np.pi * np.outer(js, e) / D)
        bm = -(2.0 / D) * np.sin(2.0 * np.pi * np.outer(js, e) / D)
        if c == 0:
            am[0, :] = 0.0
            bm[0, :] = 0.0
        ams.append(am.astype(BF))
        bms.append(bm.astype(BF))
    return ams, bms


def _two_term_fp8(x):
    x1 = x.astype(F8NP)
    x2 = (x - x1.astype(x.dtype)).astype(F8NP)
    return x1, x2


def _run(inputs, trace=False):
    if "nc" not in _CACHE:
        _CACHE["nc"] = _build()
    nc = _CACHE["nc"]
    ams, bms = _CACHE.setdefault("const", _constants())

    h32 = np.asarray(inputs["hidden_states"], np.float32).reshape(R, D)
    hT = np.ascontiguousarray(h32.T)                     # [D, R] f32
    h1, h2 = _two_term_fp8(hT)
    base = np.ascontiguousarray(
        np.asarray(inputs["base_output"], np.float32).reshape(R, D))
    gate = np.asarray(inputs["gate"], np.float32).reshape(-1)[0]

    # Host-side fold of the DFT into the projections via rfft of W.T
    # (exact):  G[:, core c bins] = [Re(F[:, js]) | Im(F[:, js])] where
    # F = rfft(W.T, axis=1); Im comes from the -sin convention.
    gf1, gf2 = {}, {}
    for x in "kvq":
        w = np.asarray(inputs[f"W{x}"], np.float64)
        F = np.fft.rfft(w.T, axis=1)                     # [D, FP+1]
        g = np.empty((D, 2 * FP), np.float64)
        blocks = g.reshape(D, NCORES, 2, FPC)
        Fre = F.real[:, :FP].reshape(D, NCORES, FPC)
        Fim = F.imag[:, :FP].reshape(D, NCORES, FPC)
        blocks[:, :, 0, :] = Fre
        blocks[:, :, 1, :] = Fim
        blocks[:, 0, 0, 0] = 0.0                         # DC re
        blocks[:, 0, 1, 0] = 0.0                         # packed Nyquist slot
        gf1[x], gf2[x] = _two_term_fp8(g)

    bfc = np.zeros((FP, 6), np.float64)
    for j, bn in enumerate(("bk", "bv", "bq")):
        spec = np.fft.rfft(np.asarray(inputs[bn], np.float64))
        bfc[:, 2 * j] = spec.real[:FP]
        bfc[:, 2 * j + 1] = spec.imag[:FP]
        bfc[0, 2 * j] = 0.0
        bfc[0, 2 * j + 1] = 0.0
    bfc = bfc.astype(np.float32)

    # Exact host-side handling of the two real bins (DC, Nyquist): their
    # adapter contribution is rank-1 over d and is folded into base.
    h64 = np.asarray(inputs["hidden_states"], np.float64).reshape(R, D)
    sgn = np.cos(np.pi * np.arange(D))            # (-1)^d
    spec_q = np.fft.rfft(np.asarray(inputs["bq"], np.float64))
    spec_k = np.fft.rfft(np.asarray(inputs["bk"], np.float64))
    spec_v = np.fft.rfft(np.asarray(inputs["bv"], np.float64))
    w64 = {x: np.asarray(inputs[f"W{x}"], np.float64) for x in "qkv"}
    corr = np.zeros((R, D), np.float64)
    for bin_idx, fold in ((0, np.ones(D)), (FP, sgn)):
        gq = w64["q"].T @ fold
        gk = w64["k"].T @ fold
        gv = w64["v"].T @ fold
        qb = h64 @ gq + (spec_q.real[bin_idx])
        kb = h64 @ gk + (spec_k.real[bin_idx])
        vb = h64 @ gv + (spec_v.real[bin_idx])
        kb = kb / (np.abs(kb) + EPS)
        vb = vb / (np.abs(vb) + EPS)
        mem = np.cumsum((kb * vb).reshape(B, S), axis=1).reshape(R)
        ob = qb * mem / D                          # w=1 for real bins
        corr += np.outer(ob, fold)
    gate64 = float(np.asarray(inputs["gate"], np.float64).reshape(-1)[0])
    base = base + (gate64 * corr).astype(np.float32)

    in_maps = []
    for c in range(NCORES):
        sl = slice(c * 2 * FPC, (c + 1) * 2 * FPC)
        im = {
            "h1": h1, "h2": h2,
            "am": (ams[c].astype(np.float32) * gate).astype(BF),
            "bm": (bms[c].astype(np.float32) * gate).astype(BF),
            "bfc": np.ascontiguousarray(bfc[c * FPC:(c + 1) * FPC]),
            "base": np.ascontiguousarray(
                base[:, c * DC:(c + 1) * DC].T).astype(BF),
        }
        for x in "kvq":
            im[f"g1{x}"] = np.ascontiguousarray(gf1[x][:, sl])
            im[f"g2{x}"] = np.ascontiguousarray(gf2[x][:, sl])
        in_maps.append(im)

    res = bass_utils.run_bass_kernel_spmd(
        nc, in_maps, core_ids=list(range(NCORES)), trace=trace)
    out = np.concatenate(
        [np.asarray(res.results[c]["out"]) for c in range(NCORES)], axis=0)
    # restore the bf16 quantization of base exactly (host-side residual)
    full = np.ascontiguousarray(out.T).astype(np.float32)
    full += base - base.astype(BF).astype(np.float32)
    return full.reshape(B, S, D), res


def kernel(**inputs) -> np.ndarray:
    out, _ = _run(inputs, trace=False)
    return out
